# revision 1
# baseline (speedup 1.0000x reference)
"""Multi-head attention (B=2, N=2048, D=1024, H=16) on 8 Trainium2 cores.

Sharding: data-parallel over batch (2) x tensor-parallel over head groups (4).
Core c handles batch c//4, heads 4*(c%4) .. 4*(c%4)+3.

Per-core kernel (matmuls at full PE rate via float32r, P*V in bf16):
  front:   kT = ([Wk;bk]^T @ [x^T;1])   (channels on partitions)
           v  = ([x;1] @ [Wv;bv])       (tokens on partitions, [v|1] blocks)
  per query-tile of 512 (ACT-exp is the pacing engine, ~32us/tile):
           qT slice = ([Wq;bq]^T @ [x^T;1])
           for each key-ptile of 128:
             S^T[:,h,:] = kT_h^T qT_h    (keys on partitions, 4 single-shot
                                          matmuls into the 4 banks of one tile)
             P^T = exp(SCALE * S^T)      (one ACT op over all 4 heads)
             [O^T_h; sums_h] += [v_h|1]^T P^T_h   (per-head chain, own bank)
           O^T_h *= broadcast(1/sums_h)  (DVE recip + gpsimd bcast + DVE mul)
           out[tokens of this tile] = sum_h O^T_h^T @ Wo_h  (K=64 chains)
Host: out[b] = sum of the 4 group partials + b_o.
"""

import sys

sys.path.insert(0, "/opt/trn_rl_repo")

import numpy as np

B, N, D, H = 2, 2048, 1024, 16
SUB = D // H  # 64
GROUPS = 4  # tensor-parallel head groups
NH = H // GROUPS  # 4 local heads per core
CH = NH * SUB  # 256 local channels
NCORES = 8


def build_nc(NT=N, DK=D, DO=D, nh=NH, name="mha"):
    import concourse.mybir as mybir
    from concourse import bacc
    from concourse.tile import TileContext

    f32 = mybir.dt.float32
    f32r = mybir.dt.float32r
    bf16 = mybir.dt.bfloat16
    Exp = mybir.ActivationFunctionType.Exp
    mult = mybir.AluOpType.mult

    sub = 64
    ch = nh * sub
    KT = DK // 128  # contraction ptiles
    CHT = ch // 128  # channel ptiles
    TOKT = NT // 128  # token/key ptiles
    QT = NT // 512  # query tiles
    TPQ = TOKT // QT  # token ptiles emitted per query tile (4)
    scale = sub ** -0.5

    nc = bacc.Bacc(None, name=name)
    xT = nc.dram_tensor("xT", [DK, NT], f32r, kind="ExternalInput")
    wq = nc.dram_tensor("wq", [DK + 1, ch], f32r, kind="ExternalInput")
    wk = nc.dram_tensor("wk", [DK + 1, ch], f32r, kind="ExternalInput")
    wv = nc.dram_tensor("wv", [DK + 1, ch], f32r, kind="ExternalInput")
    wo = nc.dram_tensor("wo", [ch, DO], f32r, kind="ExternalInput")
    ones_in = nc.dram_tensor("ones", [1, 512], f32r, kind="ExternalInput")
    bq = nc.dram_tensor("bq", [ch, 1], f32, kind="ExternalInput")
    bk = nc.dram_tensor("bk", [ch, 1], f32, kind="ExternalInput")
    out = nc.dram_tensor("out", [NT, DO], f32, kind="ExternalOutput")

    with TileContext(nc) as tc:
        with tc.tile_pool(name="persist", bufs=1) as pp:
            ones = pp.tile([1, 512], f32r)
            qT_sb = pp.tile([128, CHT, NT], f32r)
            kT_sb = pp.tile([128, CHT, NT], f32r)
            v_sb = pp.tile([128, TOKT, ch], bf16)
            ones_c = pp.tile([128, 1], bf16)
            oT_sb = pp.tile([128, CHT, NT], f32r)
            wo_sb = pp.tile([128, CHT, DO], f32r)
            nc.sync.dma_start(ones[:], ones_in[:])
            ones_f = pp.tile([128, 1], f32)
            nc.vector.memset(ones_f[:], 1.0)
            nc.vector.tensor_copy(ones_c[:], ones_f[:])
            zeros_c = pp.tile([128, 128], bf16)
            zeros_f = pp.tile([128, 128], f32)
            nc.vector.memset(zeros_f[:], 0.0)
            nc.vector.tensor_copy(zeros_c[:], zeros_f[:])
            bqk_sb = pp.tile([128, 2, CHT], f32)
            for i, bsrc in enumerate((bq, bk)):
                for ct in range(CHT):
                    nc.sync.dma_start(
                        bqk_sb[:, i, ct : ct + 1], bsrc[ct * 128 : (ct + 1) * 128, :]
                    )
            for ct in range(CHT):
                nc.sync.dma_start(wo_sb[:, ct, :], wo[ct * 128 : (ct + 1) * 128, :])

            with tc.tile_pool(name="xp", bufs=1) as xp, \
                 tc.tile_pool(name="wp", bufs=1) as wp, \
                 tc.tile_pool(name="stp", bufs=2, space="PSUM") as stp, \
                 tc.tile_pool(name="acc", bufs=4, space="PSUM") as acc, \
                 tc.tile_pool(name="ptp", bufs=8) as ptp, \
                 tc.tile_pool(name="nrm", bufs=4) as nrm, \
                 tc.tile_pool(name="osg", bufs=4) as osg:
                xt = xp.tile([128, KT, NT], f32r)
                w_sb = {}

                def load_w(nm, dram):
                    wch = ch
                    wt = wp.tile([128, KT, wch], f32r, name=f"{nm}t", tag=f"{nm}t")
                    for kt in range(KT):
                        nc.sync.dma_start(
                            wt[:, kt, :], dram[kt * 128 : (kt + 1) * 128, :]
                        )
                    wb = wp.tile([1, wch], f32r, name=f"{nm}b", tag=f"{nm}b")
                    nc.sync.dma_start(wb[:], dram[DK : DK + 1, :])
                    w_sb[nm] = (wt, wb)

                load_w("wk", wk)
                for kt in range(KT):
                    nc.sync.dma_start(xt[:, kt, :], xT[kt * 128 : (kt + 1) * 128, :])
                load_w("wv", wv)
                load_w("wq", wq)

                def qk_proj(dst, nm, mt, qt, pool=None, tag="acc"):
                    """dst[:, mt, qt*512:+512] = (W^T @ x^T) slice + per-partition bias."""
                    wt, wb = w_sb[nm]
                    ps = (pool or acc).tile([128, 512], f32, name="ps", tag=tag)
                    for kt in range(KT):
                        nc.tensor.matmul(
                            ps[:],
                            lhsT=wt[:, kt, mt * 128 : (mt + 1) * 128],
                            rhs=xt[:, kt, qt * 512 : (qt + 1) * 512],
                            start=(kt == 0),
                            stop=(kt == KT - 1),
                        )
                    nc.vector.tensor_scalar_add(
                        dst[:, mt, qt * 512 : (qt + 1) * 512],
                        ps[:],
                        bqk_sb[:, 0 if nm == "wq" else 1, mt : mt + 1],
                    )

                def v_proj(tt, pool=None, tag="acc"):
                    """v_sb[:, tt, :] = ([x;1] @ [Wv;bv])."""
                    wt, wb = w_sb["wv"]
                    ps = (pool or acc).tile([128, ch], f32, name="psv", tag=tag)
                    for kt in range(KT):
                        nc.tensor.matmul(
                            ps[:],
                            lhsT=xt[:, kt, tt * 128 : (tt + 1) * 128],
                            rhs=wt[:, kt, :],
                            start=(kt == 0),
                            stop=False,
                        )
                    nc.tensor.matmul(
                        ps[:],
                        lhsT=ones[0:1, 0:128],
                        rhs=wb[:],
                        start=False,
                        stop=True,
                    )
                    nc.vector.tensor_copy(v_sb[:, tt, :], ps[:])

                def outproj_piece(tt, nt):
                    ps = acc.tile([128, 512], f32, name="ops", tag="acc")
                    for ct in range(CHT):
                        nc.tensor.matmul(
                            ps[:],
                            lhsT=oT_sb[:, ct, tt * 128 : (tt + 1) * 128],
                            rhs=wo_sb[:, ct, nt * 512 : (nt + 1) * 512],
                            start=(ct == 0),
                            stop=(ct == CHT - 1),
                        )
                    stg = osg.tile([128, 512], f32, name="stg", tag="stg")
                    nc.vector.tensor_copy(stg[:], ps[:])
                    nc.sync.dma_start(
                        out[tt * 128 : (tt + 1) * 128, nt * 512 : (nt + 1) * 512],
                        stg[:],
                    )
                # streamed projections: minimal front, everything else
                # trickles through the spare acc slot under the ACT-paced loop
                from collections import deque

                NVF = 2  # v tiles projected upfront; the rest stream just-in-time
                pending = deque()
                for tt in range(NVF, TOKT):
                    pending.append(("v", tt))
                for qt in range(1, QT):
                    for mt in range(CHT):
                        pending.append(("q", mt, qt))

                def emit(item):
                    kind = item[0]
                    if kind == "v":
                        v_proj(item[1])
                    elif kind == "q":
                        qk_proj(qT_sb, "wq", item[1], item[2])
                    elif kind == "o":
                        outproj_piece(item[1], item[2])

                # minimal front: all of kT, first two v tiles, qT of qt 0.
                # chains alternate between the acc slots and the (idle) stp
                # slots so six are in flight instead of four
                front = [("k", mt, qt) for mt in range(CHT) for qt in range(QT)]
                front += [("vf", tt) for tt in range(NVF)]
                front += [("qf", mt) for mt in range(CHT)]
                for i, item in enumerate(front):
                    pool, tag = (stp, "st") if i % 2 else (acc, "acc")
                    if item[0] == "k":
                        qk_proj(kT_sb, "wk", item[1], item[2], pool=pool, tag=tag)
                    elif item[0] == "vf":
                        v_proj(item[1], pool=pool, tag=tag)
                    else:
                        qk_proj(qT_sb, "wq", item[1], 0, pool=pool, tag=tag)
                for qt in range(QT):
                    ot = [
                        acc.tile([128, 512], f32, name=f"otp{p}", tag="acc")
                        for p in range(nh // 2)
                    ]
                    sm = acc.tile([97, 512], f32, name="sm", tag="acc")
                    for kt2 in range(TOKT):
                        if pending and (pending[0][0] == "v" or kt2 % 2 == 0):
                            emit(pending.popleft())
                        first, last = kt2 == 0, kt2 == TOKT - 1
                        # two half-tiles (2 heads / 2 banks each), double-buffered:
                        # exp of one half pipelines against S-matmuls of the other
                        for half in range(nh // 2):
                            st = stp.tile([128, 2, 512], f32, name="st", tag="st")
                            for hh in range(2):
                                h = 2 * half + hh
                                bp = 64 * hh
                                nc.tensor.matmul(
                                    st[:, hh, :],
                                    lhsT=kT_sb[bp : bp + 64, half, kt2 * 128 : (kt2 + 1) * 128],
                                    rhs=qT_sb[bp : bp + 64, half, qt * 512 : (qt + 1) * 512],
                                    start=True,
                                    stop=True,
                                )
                            pt = ptp.tile([128, 2, 512], bf16, name="pt", tag="pt")
                            nc.scalar.activation(pt[:], st[:], Exp, scale=scale)
                            if first and half == 0:
                                # open the shared-bank has_written groups with
                                # zero matmuls, emitted after the first S/exp so
                                # they don't head-of-line-block the PE stream on
                                # the previous qt's normalize
                                for pp_ in range(nh // 2):
                                    nc.tensor.matmul(
                                        ot[pp_][:], lhsT=zeros_c[:], rhs=v_sb[:, 0:2, :],
                                        start=True, stop=False, skip_group_check=True,
                                    )
                                nc.tensor.matmul(
                                    sm[:], lhsT=zeros_c[:, 0:97], rhs=v_sb[:, 0:2, :],
                                    start=True, stop=False, skip_group_check=True,
                                )
                            for hh in range(2):
                                h = 2 * half + hh
                                nc.tensor.matmul(
                                    ot[half][64 * hh : 64 * hh + 64, :],
                                    lhsT=v_sb[:, kt2, 64 * h : 64 * h + 64],
                                    rhs=pt[:, hh, :],
                                    start=False,
                                    stop=last,
                                    skip_group_check=True,
                                )
                                nc.tensor.matmul(
                                    sm[32 * h : 32 * h + 1, :],
                                    lhsT=ones_c[:],
                                    rhs=pt[:, hh, :],
                                    start=False,
                                    stop=last,
                                    tile_position=(0, 32 * h),
                                    skip_group_check=True,
                                )
                    for h in range(nh):
                        bp = 64 * (h % 2)
                        rcp = nrm.tile([97, 512], f32, name="rcp", tag="rcp")
                        row0 = nrm.tile([1, 512], f32, name="row0", tag="row0")
                        bc = nrm.tile([64, 512], f32, name="bc", tag="bc")
                        nc.vector.reciprocal(rcp[32 * h : 32 * h + 1, :], sm[32 * h : 32 * h + 1, :])
                        # gpsimd broadcast reads physical partition 0: stage there
                        nc.sync.dma_start(row0[:], rcp[32 * h : 32 * h + 1, :])
                        nc.gpsimd.partition_broadcast(bc[:], row0[:], channels=64)
                        nc.vector.tensor_tensor(
                            out=oT_sb[bp : bp + 64, h // 2, qt * 512 : (qt + 1) * 512],
                            in0=ot[h // 2][bp : bp + 64, :],
                            in1=bc[:],
                            op=mult,
                        )
                    for tt in range(qt * TPQ, min((qt + 1) * TPQ, TOKT)):
                        for nt in range(DO // 512):
                            pending.append(("o", tt, nt))
                while pending:
                    emit(pending.popleft())
    nc.finalize()
    return nc


def make_in_maps(x, W_qkv, b_qkv, W_o):
    """Shard full inputs into per-core input maps (core c: batch c//4, group c%4)."""
    x = np.asarray(x, dtype=np.float32)
    W_qkv = np.asarray(W_qkv, dtype=np.float32)
    b_qkv = np.asarray(b_qkv, dtype=np.float32)
    W_o = np.asarray(W_o, dtype=np.float32)
    in_maps = []
    for c in range(NCORES):
        b, g = divmod(c, GROUPS)
        cols = slice(CH * g, CH * (g + 1))
        m = {
            "xT": np.ascontiguousarray(x[b].T),
            "wq": np.ascontiguousarray(
                np.concatenate([W_qkv[:, 0 * D : 1 * D][:, cols], b_qkv[0 * D : 1 * D][cols][None, :]], 0)
            ),
            "wk": np.ascontiguousarray(
                np.concatenate([W_qkv[:, 1 * D : 2 * D][:, cols], b_qkv[1 * D : 2 * D][cols][None, :]], 0)
            ),
            "wv": np.ascontiguousarray(
                np.concatenate([W_qkv[:, 2 * D : 3 * D][:, cols], b_qkv[2 * D : 3 * D][cols][None, :]], 0)
            ),
            "wo": np.ascontiguousarray(W_o[cols, :]),
            "ones": np.ones((1, 512), dtype=np.float32),
            "bq": np.ascontiguousarray(b_qkv[0 * D : 1 * D][cols][:, None]),
            "bk": np.ascontiguousarray(b_qkv[1 * D : 2 * D][cols][:, None]),
        }
        in_maps.append(m)
    return in_maps


_NC = None


def get_nc():
    global _NC
    if _NC is None:
        _NC = build_nc()
    return _NC


def kernel(x, W_qkv, b_qkv, W_o, b_o):
    from concourse import bass_utils

    b_o = np.asarray(b_o, dtype=np.float32)
    in_maps = make_in_maps(x, W_qkv, b_qkv, W_o)
    res = bass_utils.run_bass_kernel_spmd(get_nc(), in_maps, core_ids=list(range(NCORES)))
    out = np.empty((B, N, D), dtype=np.float32)
    for b in range(B):
        acc = res.results[4 * b]["out"].copy()
        for g in range(1, GROUPS):
            acc += res.results[4 * b + g]["out"]
        out[b] = acc + b_o
    return out

